# revision 2
# baseline (speedup 1.0000x reference)
"""Causal self-attention (B=4, T=2048, C=1024, H=16) on 8 TRN2 NeuronCores. v2.

Sharding: core c -> (batch b = c//2, head-group g = c%2). Each core computes
QKV for its 8 heads of one batch, causal attention, and a partial output
projection; host sums the two head-group partials and adds b_proj.

v2 changes vs baseline:
  - attention operands in bf16 (qt/kt/vex/pt/ytq/wp): full PE rate at any
    moving width, 2x DVE modes, half the SBUF
  - causal width restriction: for the 4 diagonal-adjacent k-blocks only the
    valid q-range is computed (widths 512/384/256/128); a single shared
    [128,128] triangle mask handles the diagonal block itself
  - scores for a head-pair run as two row-tiled matmuls (tile_position
    (0,0)/(64,0)) -> concurrent on HW; exp is one ACT instruction per pair
  - softmax reciprocal broadcast on the Pool/GpSimd engine
  - projection PSUM is DMA'd straight to HBM (no DVE copy)
  - stage-1 of chunk t+1 is emission-interleaved into attention of chunk t
"""

import sys

for _p in ("/opt/trn_rl_repo", "/root/.axon_site/_ro/trn_rl_repo"):
    if _p not in sys.path:
        sys.path.insert(0, _p)

import os

import numpy as np

import concourse.bass as bass
import concourse.mybir as mybir
import concourse.tile as tile
from concourse.bass import ts
from concourse.bass_utils import run_bass_kernel_spmd

B, T, C, H, HD = 4, 2048, 1024, 16, 64
NH = 8           # heads per core
P = 128
QC = 512         # q-chunk width
NQC = T // QC    # 4
NKB = T // P     # 16 k-blocks
KO = C // P      # 8 contraction tiles for the C-dim
F32 = mybir.dt.float32
F32R = mybir.dt.float32r
BF16 = mybir.dt.bfloat16

USE_TILEPOS = os.environ.get("K2_TILEPOS", "1") == "1"
USE_CASTDMA = os.environ.get("K2_CASTDMA", "1") == "1"


def build_nc():
    nc = bass.Bass()

    xT = nc.dram_tensor("xT", [C, T], F32R, kind="ExternalInput")
    Wqk = nc.dram_tensor("Wqk", [C, 2 * NH * HD], F32R, kind="ExternalInput")
    Wv = nc.dram_tensor("Wv", [C, NH * HD], F32R, kind="ExternalInput")
    Wp = nc.dram_tensor("Wp", [NH * HD, C], BF16, kind="ExternalInput")
    bqk = nc.dram_tensor("bqk", [P, 2 * NH * HD // P], F32, kind="ExternalInput")
    bv = nc.dram_tensor("bv", [NH * HD], F32, kind="ExternalInput")
    tri = nc.dram_tensor("tri", [P, P], BF16, kind="ExternalInput")
    one = nc.dram_tensor("one", [P], BF16, kind="ExternalInput")
    yT = nc.dram_tensor("yT", [C, T], F32, kind="ExternalOutput")

    xT_t = xT[:].rearrange("(ko p) t -> p ko t", p=P)        # [128, 8, T]
    yT_t = yT[:].rearrange("(mo p) t -> p mo t", p=P)        # [128, 8, T]
    Wqk_t = Wqk[:].rearrange("(ko p) n -> p ko n", p=P)      # [128, 8, 1024]
    Wv_t = Wv[:].rearrange("(ko p) n -> p ko n", p=P)        # [128, 8, 512]
    Wp_t = Wp[:].rearrange("(ko p) n -> p ko n", p=P)        # [128, 4, 1024]

    with tile.TileContext(nc) as tc:
        with (
            tc.tile_pool(name="consts", bufs=1) as consts,
            tc.tile_pool(name="persist", bufs=1) as persist,
            tc.tile_pool(name="w1", bufs=1) as w1pool,
            tc.tile_pool(name="xt", bufs=2) as xtpool,
            tc.tile_pool(name="qt", bufs=2) as qtpool,
            tc.tile_pool(name="yt", bufs=2) as ytpool,
            tc.tile_pool(name="pt", bufs=3) as ptpool,
            tc.tile_pool(name="rec", bufs=2) as recpool,
            tc.tile_pool(name="recb", bufs=2) as rbpool,
            tc.tile_pool(name="yu", bufs=2) as yupool,
            tc.tile_pool(name="st", bufs=2) as stpool,
            tc.tile_pool(name="ps_sc", bufs=2, space="PSUM") as scpool,
            tc.tile_pool(name="ps_yac", bufs=1, space="PSUM") as yacpool,
            tc.tile_pool(name="ps_s1", bufs=2, space="PSUM") as s1pool,
        ):
            # ---- constants ----
            bqk_sb = consts.tile([P, 2 * NH * HD // P], F32)      # [128, 8]
            nc.sync.dma_start(bqk_sb[:], bqk[:])
            bv_sb = consts.tile([P, NH * HD], F32)                # [128, 512]
            nc.sync.dma_start(bv_sb[:], bass.AP(bv, 0, [[0, P], [1, NH * HD]]))
            tri_sb = consts.tile([P, P], BF16)                    # [128, 128]
            nc.sync.dma_start(tri_sb[:], tri[:])
            wp_sb = consts.tile([P, NH * HD // P, C], BF16)       # [128, 4, 1024]
            nc.sync.dma_start(wp_sb[:], Wp_t[:])

            # ---- persistent activations ----
            kt_sb = persist.tile([P, NH * HD // P, T], BF16)       # [128, 4, 2048]
            vex_sb = persist.tile([P, NKB, NH, HD + 1], BF16)      # [128,16,8,65]
            ones_sb = consts.tile([P, P], BF16)
            nc.sync.dma_start(ones_sb[:], bass.AP(one, 0, [[0, P], [1, P]]))
            nc.vector.tensor_copy(
                vex_sb[:, :, :, HD : HD + 1].rearrange("p a b c -> p (a b c)"),
                ones_sb[:],
            )
            ones_row = consts.tile([1, HD], F32R)
            nc.vector.memset(ones_row[:].bitcast(F32), 1.0)

            # ---- stage-1 weights ----
            wqk_sb = w1pool.tile([P, KO, 2 * NH * HD], F32R)       # 4MB
            nc.sync.dma_start(wqk_sb[:], Wqk_t[:])
            wv_sb = w1pool.tile([P, KO, NH * HD], F32R)            # 2MB
            nc.sync.dma_start(wv_sb[:], Wv_t[:])

            xts = [None] * NQC
            qts = [None] * NQC

            def emit_xt_dma(tci):
                xt = xtpool.tile([P, KO, QC], F32R, name=f"xt{tci}", tag="xt")
                nc.sync.dma_start(xt[:], xT_t[:, :, ts(tci, QC)])
                xts[tci] = xt
                qts[tci] = qtpool.tile(
                    [P, NH * HD // P, QC], BF16, name=f"qt{tci}", tag="qt"
                )

            def stage1_groups(tci):
                """12 emit-closures: 8 QK m-tiles + 4 V row-blocks."""
                groups = []

                def qk_group(m):
                    def emit():
                        ps = s1pool.tile([P, QC], F32, name="ps_qk", tag="s1")
                        for k in range(KO):
                            nc.tensor.matmul(
                                ps[:],
                                wqk_sb[:, k, ts(m, P)],
                                xts[tci][:, k, :],
                                start=(k == 0),
                                stop=(k == KO - 1),
                            )
                        if m < NH * HD // P:
                            dst = qts[tci][:, m, :]
                        else:
                            dst = kt_sb[:, m - NH * HD // P, ts(tci, QC)]
                        nc.vector.tensor_scalar_add(dst, ps[:], bqk_sb[:, m : m + 1])

                    return emit

                def v_group(t4):
                    def emit():
                        kb = tci * (QC // P) + t4
                        psv = s1pool.tile([P, NH * HD], F32, name="ps_v", tag="s1")
                        for k in range(KO):
                            nc.tensor.matmul(
                                psv[:],
                                xts[tci][:, k, ts(t4, P)],
                                wv_sb[:, k, :],
                                start=(k == 0),
                                stop=(k == KO - 1),
                            )
                        nc.vector.tensor_add(
                            vex_sb[:, kb, :, :HD],
                            psv[:].rearrange("p (h d) -> p h d", h=NH),
                            bv_sb[:].rearrange("p (h d) -> p h d", h=NH),
                        )

                    return emit

                for m in range(2 * NH * HD // P):
                    groups.append(qk_group(m))
                for t4 in range(QC // P):
                    groups.append(v_group(t4))
                return groups

            def emit_pair(tci, pr):
                """Attention for head pair (2*pr, 2*pr+1) of q-chunk tci."""
                qt = qts[tci]
                nkb = (tci + 1) * (QC // P)
                yac = yacpool.tile([P, 2, QC], F32, name="yac", tag="yac")
                pend = []  # deferred attV emits: (kb, q0, pt_tile)

                def attv(kb, q0, pt):
                    for pi in range(2):
                        nc.tensor.matmul(
                            yac[0 : HD + 1, pi, q0:QC],
                            vex_sb[:, kb, 2 * pr + pi, :],
                            pt[:, pi, q0:QC],
                            start=(kb == 0),
                            stop=(kb == nkb - 1),
                        )

                for kb in range(nkb):
                    d = kb - tci * (QC // P)
                    q0 = max(d, 0) * P
                    spair = scpool.tile([P, 2, QC], F32, name="spair", tag="sc")
                    for pi in range(2):
                        nc.tensor.matmul(
                            spair[:, pi, q0:QC],
                            kt_sb[ts(pi, HD), pr, ts(kb, P)],
                            qt[ts(pi, HD), pr, q0:QC],
                            start=True,
                            stop=True,
                            tile_position=(pi * HD, 0) if USE_TILEPOS else None,
                        )
                    pt = ptpool.tile([P, 2, QC], BF16, name="pt", tag="pt")
                    nc.scalar.activation(
                        pt[:, :, q0:QC],
                        spair[:, :, q0:QC],
                        mybir.ActivationFunctionType.Exp,
                        scale=1.0 / np.sqrt(HD),
                    )
                    if d >= 0:
                        for pi in range(2):
                            nc.vector.tensor_mul(
                                pt[:, pi, q0 : q0 + P],
                                pt[:, pi, q0 : q0 + P],
                                tri_sb[:],
                            )
                    pend.append((kb, q0, pt))
                    if len(pend) >= 2:
                        attv(*pend.pop(0))
                while pend:
                    attv(*pend.pop(0))

                # softmax normalization (baseline-proven pattern, per head):
                # reciprocal of the denominator row, PE ones-matmul broadcast
                # into a rotating s1 PSUM bank, evacuate, multiply.
                recb_ps = scpool.tile([P, 2, QC], F32, name="recb_ps", tag="sc")
                for pi in range(2):
                    rec = recpool.tile([1, QC], F32, name="rec", tag="rec")
                    nc.vector.reciprocal(rec[:], yac[HD : HD + 1, pi, :])
                    rec_r = recpool.tile([1, QC], F32R, name="rec_r", tag="rec_r")
                    nc.vector.tensor_copy(rec_r[:], rec[:])
                    nc.tensor.matmul(
                        recb_ps[0:HD, pi, :], ones_row[:], rec_r[:],
                        start=True, stop=True,
                    )
                for pi in range(2):
                    recb = rbpool.tile([HD, QC], F32, name="recb", tag="recb")
                    nc.vector.tensor_copy(recb[:], recb_ps[0:HD, pi, :])
                    nc.vector.tensor_mul(
                        ytqs[tci][ts(pi, HD), pr, :],
                        yac[0:HD, pi, :],
                        recb[:],
                    )

            ytqs = [None] * NQC

            def proj_groups(tci):
                groups = []

                def proj_m(m):
                    def emit():
                        pp = s1pool.tile([P, QC], F32, name="pp", tag="s1")
                        for kk in range(NH * HD // P):
                            nc.tensor.matmul(
                                pp[:],
                                wp_sb[:, kk, ts(m, P)],
                                ytqs[tci][:, kk, :],
                                start=(kk == 0),
                                stop=(kk == NH * HD // P - 1),
                            )
                        st = stpool.tile([P, QC], F32, name="st", tag="st")
                        nc.vector.tensor_copy(st[:], pp[:])
                        nc.sync.dma_start(yT_t[:, m, ts(tci, QC)], st[:])

                    return emit

                return [proj_m(m) for m in range(C // P)]

            # ---------------- schedule ----------------
            emit_xt_dma(0)
            for g in stage1_groups(0):
                g()
            for tci in range(NQC):
                ytqs[tci] = ytpool.tile(
                    [P, NH * HD // P, QC], BF16, name=f"ytq{tci}", tag="ytq"
                )
                fillers = []
                if tci + 1 < NQC:
                    emit_xt_dma(tci + 1)
                    fillers += stage1_groups(tci + 1)
                if tci > 0:
                    fillers += proj_groups(tci - 1)
                npr = NH // 2
                for pr in range(npr):
                    emit_pair(tci, pr)
                    lo = (len(fillers) * pr) // npr
                    hi = (len(fillers) * (pr + 1)) // npr
                    for g in fillers[lo:hi]:
                        g()
            for g in proj_groups(NQC - 1):
                g()

    return nc


def legalize_waits(nc):
    """This walrus build accepts at most 1 sync wait per instruction (0 for
    self-loading fp32/fp32r Matmult, whose LW slot takes none). Move excess
    waits onto preceding same-engine NoOps; engines execute in order so the
    guarantee is identical."""
    n = 0
    for blk in nc.m.functions[0].blocks:
        new = []
        for inst in blk.instructions:
            si = inst.sync_info
            waits = list(si.on_wait) if si is not None and si.on_wait else []
            lim = 0 if inst.opcode in ("Matmult", "Ldweights") else 1
            if len(waits) > lim:
                keep = waits[len(waits) - lim:] if lim else []
                for w in waits[: len(waits) - lim]:
                    n += 1
                    new.append(mybir.InstNoOp(
                        name=f"I-wfix{n}", engine=inst.engine, ins=[], outs=[],
                        sync_info=mybir.SyncInfo(on_wait=[w], on_update=[]),
                    ))
                inst.sync_info = mybir.SyncInfo(
                    on_wait=keep,
                    on_update=list(si.on_update) if si.on_update else [],
                )
            new.append(inst)
        blk.instructions = new
    return n


def _host_inputs(x, W_attn, b_attn, W_proj):
    """Build the 8 per-core input maps."""
    import ml_dtypes

    kl = np.arange(P)[:, None]
    ql = np.arange(P)[None, :]
    tri = (ql >= kl).astype(ml_dtypes.bfloat16)

    in_maps = []
    for core in range(8):
        b, g = core // 2, core % 2
        qs = slice(g * NH * HD, (g + 1) * NH * HD)
        ks = slice(C + g * NH * HD, C + (g + 1) * NH * HD)
        vs = slice(2 * C + g * NH * HD, 2 * C + (g + 1) * NH * HD)
        wqk = np.ascontiguousarray(
            np.concatenate([W_attn[:, qs], W_attn[:, ks]], axis=1)
        )
        bqk = (
            np.concatenate([b_attn[qs], b_attn[ks]])
            .reshape(2 * NH * HD // P, P)
            .T.copy()
        )
        in_maps.append(
            {
                "xT": np.ascontiguousarray(x[b].T),
                "Wqk": wqk,
                "Wv": np.ascontiguousarray(W_attn[:, vs]),
                "Wp": np.ascontiguousarray(
                    W_proj[g * NH * HD : (g + 1) * NH * HD]
                ).astype(ml_dtypes.bfloat16),
                "bqk": np.ascontiguousarray(bqk),
                "bv": np.ascontiguousarray(b_attn[vs]),
                "tri": tri,
                "one": np.ones([P], dtype=ml_dtypes.bfloat16),
            }
        )
    return in_maps


def run(x, W_attn, b_attn, W_proj, b_proj, trace=False):
    """Returns (y, BassKernelResults)."""
    x = np.asarray(x, dtype=np.float32)
    W_attn = np.asarray(W_attn, dtype=np.float32)
    b_attn = np.asarray(b_attn, dtype=np.float32)
    W_proj = np.asarray(W_proj, dtype=np.float32)
    b_proj = np.asarray(b_proj, dtype=np.float32)

    nc = build_nc()
    if os.environ.get("K2_NOLEGALIZE", "0") != "1":
        legalize_waits(nc)
    in_maps = _host_inputs(x, W_attn, b_attn, W_proj)
    res = run_bass_kernel_spmd(nc, in_maps, list(range(8)), trace=trace)

    y = np.empty((B, T, C), dtype=np.float32)
    for b in range(B):
        acc = res.results[2 * b]["yT"] + res.results[2 * b + 1]["yT"]
        y[b] = acc.T + b_proj
    return y, res


def kernel(x, W_attn, b_attn, W_proj, b_proj):
    y, _ = run(x, W_attn, b_attn, W_proj, b_proj)
    return y


# revision 4
# speedup vs baseline: 215.8056x; 215.8056x over previous
"""Causal self-attention (B=4, T=2048, C=1024, H=16) on 8 TRN2 NeuronCores. v2.

Sharding: core c -> (batch b = c//2, head-group g = c%2). Each core computes
QKV for its 8 heads of one batch, causal attention, and a partial output
projection; host sums the two head-group partials and adds b_proj.

v2 changes vs the original baseline:
  - attention operands in bf16 (qt/kt/vex/pt/ytq/wp): full PE rate at any
    moving width, 2x DVE perf modes, half the SBUF footprint
  - causal width restriction: for the 4 diagonal-adjacent k-blocks only the
    valid q-range is computed (widths 512/384/256/128); a single shared
    [128,128] triangle mask handles the diagonal block itself
  - scores for a head-pair run as two row-group matmuls (tile_position
    (0,0)/(64,0)) writing two PSUM banks -> concurrent on HW; exp is ONE
    ACT instruction per pair spanning both banks (amortizes ACT overhead)
  - softmax normalization per head: reciprocal of the fused denominator row
    (ones column of vex), PE ones-matmul broadcast into a rotating scores
    PSUM bank, evacuate, multiply (all HW-proven op patterns)
  - stage-1 of chunk t+1 and projection of chunk t-1 are emission-interleaved
    between attention head-pairs of chunk t to fill ACT-paced PE gaps
"""

import sys

for _p in ("/opt/trn_rl_repo", "/root/.axon_site/_ro/trn_rl_repo"):
    if _p not in sys.path:
        sys.path.insert(0, _p)

import os

import numpy as np

import concourse.bass as bass
import concourse.mybir as mybir
import concourse.tile as tile
from concourse.bass import ts
from concourse.bass_utils import run_bass_kernel_spmd

B, T, C, H, HD = 4, 2048, 1024, 16, 64
NH = 8           # heads per core
P = 128
QC = 512         # q-chunk width
NQC = T // QC    # 4
NKB = T // P     # 16 k-blocks
KO = C // P      # 8 contraction tiles for the C-dim
F32 = mybir.dt.float32
F32R = mybir.dt.float32r
BF16 = mybir.dt.bfloat16

USE_TILEPOS = os.environ.get("K2_TILEPOS", "1") == "1"


def build_nc():
    nc = bass.Bass()

    xT = nc.dram_tensor("xT", [C, T], F32R, kind="ExternalInput")
    Wqk = nc.dram_tensor("Wqk", [C, 2 * NH * HD], F32R, kind="ExternalInput")
    Wv = nc.dram_tensor("Wv", [C, NH * HD], F32R, kind="ExternalInput")
    Wp = nc.dram_tensor("Wp", [NH * HD, C], BF16, kind="ExternalInput")
    bqk = nc.dram_tensor("bqk", [P, 2 * NH * HD // P], F32, kind="ExternalInput")
    bv = nc.dram_tensor("bv", [NH * HD], F32, kind="ExternalInput")
    tri = nc.dram_tensor("tri", [P, P], BF16, kind="ExternalInput")
    one = nc.dram_tensor("one", [P], BF16, kind="ExternalInput")
    yT = nc.dram_tensor("yT", [C, T], F32, kind="ExternalOutput")

    xT_t = xT[:].rearrange("(ko p) t -> p ko t", p=P)        # [128, 8, T]
    yT_t = yT[:].rearrange("(mo p) t -> p mo t", p=P)        # [128, 8, T]
    Wqk_t = Wqk[:].rearrange("(ko p) n -> p ko n", p=P)      # [128, 8, 1024]
    Wv_t = Wv[:].rearrange("(ko p) n -> p ko n", p=P)        # [128, 8, 512]
    Wp_t = Wp[:].rearrange("(ko p) n -> p ko n", p=P)        # [128, 4, 1024]

    with tile.TileContext(nc) as tc:
        with (
            tc.tile_pool(name="consts", bufs=1) as consts,
            tc.tile_pool(name="persist", bufs=1) as persist,
            tc.tile_pool(name="w1", bufs=1) as w1pool,
            tc.tile_pool(name="xt", bufs=2) as xtpool,
            tc.tile_pool(name="qt", bufs=2) as qtpool,
            tc.tile_pool(name="yt", bufs=2) as ytpool,
            tc.tile_pool(name="pt", bufs=3) as ptpool,
            tc.tile_pool(name="rec", bufs=2) as recpool,
            tc.tile_pool(name="recb", bufs=2) as rbpool,
            tc.tile_pool(name="st", bufs=2) as stpool,
            tc.tile_pool(name="ps_sc", bufs=2, space="PSUM") as scpool,
            tc.tile_pool(name="ps_yac", bufs=1, space="PSUM") as yacpool,
            tc.tile_pool(name="ps_s1", bufs=2, space="PSUM") as s1pool,
        ):
            # ---- constants ----
            bqk_sb = consts.tile([P, 2 * NH * HD // P], F32)      # [128, 8]
            nc.sync.dma_start(bqk_sb[:], bqk[:])
            bv_sb = consts.tile([P, NH * HD], F32)                # [128, 512]
            nc.sync.dma_start(bv_sb[:], bass.AP(bv, 0, [[0, P], [1, NH * HD]]))
            tri_sb = consts.tile([P, P], BF16)                    # [128, 128]
            nc.sync.dma_start(tri_sb[:], tri[:])
            wp_sb = consts.tile([P, NH * HD // P, C], BF16)       # [128, 4, 1024]
            nc.sync.dma_start(wp_sb[:], Wp_t[:])

            # ---- persistent activations ----
            kt_sb = persist.tile([P, NH * HD // P, T], BF16)       # [128, 4, 2048]
            vex_sb = persist.tile([P, NKB, NH, HD + 1], BF16)      # [128,16,8,65]
            ones_sb = consts.tile([P, P], BF16)
            nc.sync.dma_start(ones_sb[:], bass.AP(one, 0, [[0, P], [1, P]]))
            nc.vector.tensor_copy(
                vex_sb[:, :, :, HD : HD + 1].rearrange("p a b c -> p (a b c)"),
                ones_sb[:],
            )
            ones_row = consts.tile([1, HD], F32R)
            nc.vector.memset(ones_row[:].bitcast(F32), 1.0)

            # ---- stage-1 weights ----
            wqk_sb = w1pool.tile([P, KO, 2 * NH * HD], F32R)       # 4MB
            nc.sync.dma_start(wqk_sb[:], Wqk_t[:])
            wv_sb = w1pool.tile([P, KO, NH * HD], F32R)            # 2MB
            nc.sync.dma_start(wv_sb[:], Wv_t[:])

            xts = [None] * NQC
            qts = [None] * NQC

            def emit_xt_dma(tci):
                xt = xtpool.tile([P, KO, QC], F32R, name=f"xt{tci}", tag="xt")
                nc.sync.dma_start(xt[:], xT_t[:, :, ts(tci, QC)])
                xts[tci] = xt
                qts[tci] = qtpool.tile(
                    [P, NH * HD // P, QC], BF16, name=f"qt{tci}", tag="qt"
                )

            def stage1_groups(tci):
                """12 emit-closures: 8 QK m-tiles + 4 V row-blocks."""
                groups = []

                def qk_group(m):
                    def emit():
                        ps = s1pool.tile([P, QC], F32, name="ps_qk", tag="s1")
                        for k in range(KO):
                            nc.tensor.matmul(
                                ps[:],
                                wqk_sb[:, k, ts(m, P)],
                                xts[tci][:, k, :],
                                start=(k == 0),
                                stop=(k == KO - 1),
                            )
                        if m < NH * HD // P:
                            dst = qts[tci][:, m, :]
                        else:
                            dst = kt_sb[:, m - NH * HD // P, ts(tci, QC)]
                        nc.vector.tensor_scalar_add(dst, ps[:], bqk_sb[:, m : m + 1])

                    return emit

                def v_group(t4):
                    def emit():
                        kb = tci * (QC // P) + t4
                        psv = s1pool.tile([P, NH * HD], F32, name="ps_v", tag="s1")
                        for k in range(KO):
                            nc.tensor.matmul(
                                psv[:],
                                xts[tci][:, k, ts(t4, P)],
                                wv_sb[:, k, :],
                                start=(k == 0),
                                stop=(k == KO - 1),
                            )
                        nc.vector.tensor_add(
                            vex_sb[:, kb, :, :HD],
                            psv[:].rearrange("p (h d) -> p h d", h=NH),
                            bv_sb[:].rearrange("p (h d) -> p h d", h=NH),
                        )

                    return emit

                for m in range(2 * NH * HD // P):
                    groups.append(qk_group(m))
                for t4 in range(QC // P):
                    groups.append(v_group(t4))
                return groups

            def emit_pair(tci, pr):
                """Attention for head pair (2*pr, 2*pr+1) of q-chunk tci."""
                qt = qts[tci]
                nkb = (tci + 1) * (QC // P)
                yac = yacpool.tile([P, 2, QC], F32, name="yac", tag="yac")
                pend = []  # deferred attV emits: (kb, q0, pt_tile)

                def attv(kb, q0, pt):
                    for pi in range(2):
                        nc.tensor.matmul(
                            yac[0 : HD + 1, pi, q0:QC],
                            vex_sb[:, kb, 2 * pr + pi, :],
                            pt[:, pi, q0:QC],
                            start=(kb == 0),
                            stop=(kb == nkb - 1),
                        )

                for kb in range(nkb):
                    d = kb - tci * (QC // P)
                    q0 = max(d, 0) * P
                    spair = scpool.tile([P, 2, QC], F32, name="spair", tag="sc")
                    for pi in range(2):
                        nc.tensor.matmul(
                            spair[:, pi, q0:QC],
                            kt_sb[ts(pi, HD), pr, ts(kb, P)],
                            qt[ts(pi, HD), pr, q0:QC],
                            start=True,
                            stop=True,
                            tile_position=(pi * HD, 0) if USE_TILEPOS else None,
                        )
                    pt = ptpool.tile([P, 2, QC], BF16, name="pt", tag="pt")
                    nc.scalar.activation(
                        pt[:, :, q0:QC],
                        spair[:, :, q0:QC],
                        mybir.ActivationFunctionType.Exp,
                        scale=1.0 / np.sqrt(HD),
                    )
                    if d >= 0:
                        for pi in range(2):
                            nc.vector.tensor_mul(
                                pt[:, pi, q0 : q0 + P],
                                pt[:, pi, q0 : q0 + P],
                                tri_sb[:],
                            )
                    pend.append((kb, q0, pt))
                    if len(pend) >= 2:
                        attv(*pend.pop(0))
                while pend:
                    attv(*pend.pop(0))

                # softmax normalization (baseline-proven pattern, per head):
                # reciprocal of the denominator row, PE ones-matmul broadcast
                # into a rotating s1 PSUM bank, evacuate, multiply.
                recb_ps = scpool.tile([P, 2, QC], F32, name="recb_ps", tag="sc")
                for pi in range(2):
                    rec = recpool.tile([1, QC], F32, name="rec", tag="rec")
                    nc.vector.reciprocal(rec[:], yac[HD : HD + 1, pi, :])
                    rec_r = recpool.tile([1, QC], F32R, name="rec_r", tag="rec_r")
                    nc.vector.tensor_copy(rec_r[:], rec[:])
                    nc.tensor.matmul(
                        recb_ps[0:HD, pi, :], ones_row[:], rec_r[:],
                        start=True, stop=True,
                    )
                for pi in range(2):
                    recb = rbpool.tile([HD, QC], F32, name="recb", tag="recb")
                    nc.vector.tensor_copy(recb[:], recb_ps[0:HD, pi, :])
                    nc.vector.tensor_mul(
                        ytqs[tci][ts(pi, HD), pr, :],
                        yac[0:HD, pi, :],
                        recb[:],
                    )

            ytqs = [None] * NQC

            def proj_groups(tci):
                groups = []

                def proj_m(m):
                    def emit():
                        pp = s1pool.tile([P, QC], F32, name="pp", tag="s1")
                        for kk in range(NH * HD // P):
                            nc.tensor.matmul(
                                pp[:],
                                wp_sb[:, kk, ts(m, P)],
                                ytqs[tci][:, kk, :],
                                start=(kk == 0),
                                stop=(kk == NH * HD // P - 1),
                            )
                        st = stpool.tile([P, QC], F32, name="st", tag="st")
                        nc.vector.tensor_copy(st[:], pp[:])
                        nc.sync.dma_start(yT_t[:, m, ts(tci, QC)], st[:])

                    return emit

                return [proj_m(m) for m in range(C // P)]

            # ---------------- schedule ----------------
            emit_xt_dma(0)
            for g in stage1_groups(0):
                g()
            for tci in range(NQC):
                ytqs[tci] = ytpool.tile(
                    [P, NH * HD // P, QC], BF16, name=f"ytq{tci}", tag="ytq"
                )
                fillers = []
                if tci + 1 < NQC:
                    emit_xt_dma(tci + 1)
                    fillers += stage1_groups(tci + 1)
                if tci > 0:
                    fillers += proj_groups(tci - 1)
                npr = NH // 2
                for pr in range(npr):
                    emit_pair(tci, pr)
                    lo = (len(fillers) * pr) // npr
                    hi = (len(fillers) * (pr + 1)) // npr
                    for g in fillers[lo:hi]:
                        g()
            for g in proj_groups(NQC - 1):
                g()

    return nc


def legalize_waits(nc):
    """This walrus build accepts at most 1 sync wait per instruction (0 for
    self-loading fp32/fp32r Matmult, whose LW slot takes none). Move excess
    waits onto preceding same-engine NoOps; engines execute in order so the
    guarantee is identical."""
    n = 0
    for blk in nc.m.functions[0].blocks:
        new = []
        for inst in blk.instructions:
            si = inst.sync_info
            waits = list(si.on_wait) if si is not None and si.on_wait else []
            lim = 0 if inst.opcode in ("Matmult", "Ldweights") else 1
            if len(waits) > lim:
                keep = waits[len(waits) - lim:] if lim else []
                for w in waits[: len(waits) - lim]:
                    n += 1
                    new.append(mybir.InstNoOp(
                        name=f"I-wfix{n}", engine=inst.engine, ins=[], outs=[],
                        sync_info=mybir.SyncInfo(on_wait=[w], on_update=[]),
                    ))
                inst.sync_info = mybir.SyncInfo(
                    on_wait=keep,
                    on_update=list(si.on_update) if si.on_update else [],
                )
            new.append(inst)
        blk.instructions = new
    return n


def _host_inputs(x, W_attn, b_attn, W_proj):
    """Build the 8 per-core input maps."""
    import ml_dtypes

    kl = np.arange(P)[:, None]
    ql = np.arange(P)[None, :]
    tri = (ql >= kl).astype(ml_dtypes.bfloat16)

    in_maps = []
    for core in range(8):
        b, g = core // 2, core % 2
        qs = slice(g * NH * HD, (g + 1) * NH * HD)
        ks = slice(C + g * NH * HD, C + (g + 1) * NH * HD)
        vs = slice(2 * C + g * NH * HD, 2 * C + (g + 1) * NH * HD)
        wqk = np.ascontiguousarray(
            np.concatenate([W_attn[:, qs], W_attn[:, ks]], axis=1)
        )
        bqk = (
            np.concatenate([b_attn[qs], b_attn[ks]])
            .reshape(2 * NH * HD // P, P)
            .T.copy()
        )
        in_maps.append(
            {
                "xT": np.ascontiguousarray(x[b].T),
                "Wqk": wqk,
                "Wv": np.ascontiguousarray(W_attn[:, vs]),
                "Wp": np.ascontiguousarray(
                    W_proj[g * NH * HD : (g + 1) * NH * HD]
                ).astype(ml_dtypes.bfloat16),
                "bqk": np.ascontiguousarray(bqk),
                "bv": np.ascontiguousarray(b_attn[vs]),
                "tri": tri,
                "one": np.ones([P], dtype=ml_dtypes.bfloat16),
            }
        )
    return in_maps


def run(x, W_attn, b_attn, W_proj, b_proj, trace=False):
    """Returns (y, BassKernelResults)."""
    x = np.asarray(x, dtype=np.float32)
    W_attn = np.asarray(W_attn, dtype=np.float32)
    b_attn = np.asarray(b_attn, dtype=np.float32)
    W_proj = np.asarray(W_proj, dtype=np.float32)
    b_proj = np.asarray(b_proj, dtype=np.float32)

    nc = build_nc()
    if os.environ.get("K2_NOLEGALIZE", "0") != "1":
        legalize_waits(nc)
    in_maps = _host_inputs(x, W_attn, b_attn, W_proj)
    res = run_bass_kernel_spmd(nc, in_maps, list(range(8)), trace=trace)

    y = np.empty((B, T, C), dtype=np.float32)
    for b in range(B):
        acc = res.results[2 * b]["yT"] + res.results[2 * b + 1]["yT"]
        y[b] = acc.T + b_proj
    return y, res


def kernel(x, W_attn, b_attn, W_proj, b_proj):
    y, _ = run(x, W_attn, b_attn, W_proj, b_proj)
    return y


# revision 5
# speedup vs baseline: 218.0169x; 1.0102x over previous
"""Causal self-attention (B=4, T=2048, C=1024, H=16) on 8 TRN2 NeuronCores. v2.

Sharding: core c -> (batch b = c//2, head-group g = c%2). Each core computes
QKV for its 8 heads of one batch, causal attention, and a partial output
projection; host sums the two head-group partials and adds b_proj.

v2 changes vs the original baseline:
  - attention operands in bf16 (qt/kt/vex/pt/ytq/wp): full PE rate at any
    moving width, 2x DVE perf modes, half the SBUF footprint
  - causal width restriction: for the 4 diagonal-adjacent k-blocks only the
    valid q-range is computed (widths 512/384/256/128); a single shared
    [128,128] triangle mask handles the diagonal block itself
  - scores for a head-pair run as two row-group matmuls (tile_position
    (0,0)/(64,0)) writing two PSUM banks -> concurrent on HW; exp is ONE
    ACT instruction per pair spanning both banks (amortizes ACT overhead)
  - softmax normalization per head: reciprocal of the fused denominator row
    (ones column of vex), PE ones-matmul broadcast into a rotating scores
    PSUM bank, evacuate, multiply (all HW-proven op patterns)
  - stage-1 of chunk t+1 and projection of chunk t-1 are emission-interleaved
    between attention head-pairs of chunk t to fill ACT-paced PE gaps
"""

import sys

for _p in ("/opt/trn_rl_repo", "/root/.axon_site/_ro/trn_rl_repo"):
    if _p not in sys.path:
        sys.path.insert(0, _p)

import os

import numpy as np

import concourse.bass as bass
import concourse.mybir as mybir
import concourse.tile as tile
from concourse.bass import ts
from concourse.bass_utils import run_bass_kernel_spmd

B, T, C, H, HD = 4, 2048, 1024, 16, 64
NH = 8           # heads per core
P = 128
QC = 512         # q-chunk width
NQC = T // QC    # 4
NKB = T // P     # 16 k-blocks
KO = C // P      # 8 contraction tiles for the C-dim
F32 = mybir.dt.float32
F32R = mybir.dt.float32r
BF16 = mybir.dt.bfloat16

USE_TILEPOS = os.environ.get("K2_TILEPOS", "1") == "1"


def build_nc():
    nc = bass.Bass()

    xT = nc.dram_tensor("xT", [C, T], F32R, kind="ExternalInput")
    Wqk = nc.dram_tensor("Wqk", [C, 2 * NH * HD], F32R, kind="ExternalInput")
    Wv = nc.dram_tensor("Wv", [C, NH * HD], F32R, kind="ExternalInput")
    Wp = nc.dram_tensor("Wp", [NH * HD, C], BF16, kind="ExternalInput")
    bqk = nc.dram_tensor("bqk", [P, 2 * NH * HD // P], F32, kind="ExternalInput")
    bv = nc.dram_tensor("bv", [NH * HD], F32, kind="ExternalInput")
    tri = nc.dram_tensor("tri", [P, P], BF16, kind="ExternalInput")
    one = nc.dram_tensor("one", [P], BF16, kind="ExternalInput")
    yT = nc.dram_tensor("yT", [C, T], F32, kind="ExternalOutput")

    xT_t = xT[:].rearrange("(ko p) t -> p ko t", p=P)        # [128, 8, T]
    yT_t = yT[:].rearrange("(mo p) t -> p mo t", p=P)        # [128, 8, T]
    Wqk_t = Wqk[:].rearrange("(ko p) n -> p ko n", p=P)      # [128, 8, 1024]
    Wv_t = Wv[:].rearrange("(ko p) n -> p ko n", p=P)        # [128, 8, 512]
    Wp_t = Wp[:].rearrange("(ko p) n -> p ko n", p=P)        # [128, 4, 1024]

    with tile.TileContext(nc) as tc:
        with (
            tc.tile_pool(name="consts", bufs=1) as consts,
            tc.tile_pool(name="persist", bufs=1) as persist,
            tc.tile_pool(name="w1", bufs=1) as w1pool,
            tc.tile_pool(name="xt", bufs=2) as xtpool,
            tc.tile_pool(name="qt", bufs=2) as qtpool,
            tc.tile_pool(name="yt", bufs=2) as ytpool,
            tc.tile_pool(name="pt", bufs=3) as ptpool,
            tc.tile_pool(name="rec", bufs=2) as recpool,
            tc.tile_pool(name="recb", bufs=2) as rbpool,
            tc.tile_pool(name="st", bufs=2) as stpool,
            tc.tile_pool(name="ps_sc", bufs=2, space="PSUM") as scpool,
            tc.tile_pool(name="ps_yac", bufs=1, space="PSUM") as yacpool,
            tc.tile_pool(name="ps_s1", bufs=2, space="PSUM") as s1pool,
        ):
            # ---- constants ----
            bqk_sb = consts.tile([P, 2 * NH * HD // P], F32)      # [128, 8]
            nc.sync.dma_start(bqk_sb[:], bqk[:])
            bv_sb = consts.tile([P, NH * HD], F32)                # [128, 512]
            nc.sync.dma_start(bv_sb[:], bass.AP(bv, 0, [[0, P], [1, NH * HD]]))
            tri_sb = consts.tile([P, P], BF16)                    # [128, 128]
            nc.sync.dma_start(tri_sb[:], tri[:])
            wp_sb = consts.tile([P, NH * HD // P, C], BF16)       # [128, 4, 1024]
            nc.sync.dma_start(wp_sb[:], Wp_t[:])

            # ---- persistent activations ----
            kt_sb = persist.tile([P, NH * HD // P, T], BF16)       # [128, 4, 2048]
            vex_sb = persist.tile([P, NKB, NH, HD + 1], BF16)      # [128,16,8,65]
            ones_sb = consts.tile([P, P], BF16)
            nc.sync.dma_start(ones_sb[:], bass.AP(one, 0, [[0, P], [1, P]]))
            nc.vector.tensor_copy(
                vex_sb[:, :, :, HD : HD + 1].rearrange("p a b c -> p (a b c)"),
                ones_sb[:],
            )
            ones_row = consts.tile([1, HD], F32R)
            nc.vector.memset(ones_row[:].bitcast(F32), 1.0)

            # ---- stage-1 weights ----
            wqk_sb = w1pool.tile([P, KO, 2 * NH * HD], F32R)       # 4MB
            nc.sync.dma_start(wqk_sb[:], Wqk_t[:])
            wv_sb = w1pool.tile([P, KO, NH * HD], F32R)            # 2MB
            nc.sync.dma_start(wv_sb[:], Wv_t[:])

            xts = [None] * NQC
            qts = [None] * NQC

            def emit_xt_dma(tci):
                xt = xtpool.tile([P, KO, QC], F32R, name=f"xt{tci}", tag="xt")
                nc.sync.dma_start(xt[:], xT_t[:, :, ts(tci, QC)])
                xts[tci] = xt
                qts[tci] = qtpool.tile(
                    [P, NH * HD // P, QC], BF16, name=f"qt{tci}", tag="qt"
                )

            def stage1_groups(tci):
                """12 emit-closures: 8 QK m-tiles + 4 V row-blocks."""
                groups = []

                def qk_group(m):
                    def emit():
                        ps = s1pool.tile([P, QC], F32, name="ps_qk", tag="s1")
                        for k in range(KO):
                            nc.tensor.matmul(
                                ps[:],
                                wqk_sb[:, k, ts(m, P)],
                                xts[tci][:, k, :],
                                start=(k == 0),
                                stop=(k == KO - 1),
                            )
                        if m < NH * HD // P:
                            dst = qts[tci][:, m, :]
                        else:
                            dst = kt_sb[:, m - NH * HD // P, ts(tci, QC)]
                        nc.vector.tensor_scalar_add(dst, ps[:], bqk_sb[:, m : m + 1])

                    return emit

                def v_group(t4):
                    def emit():
                        kb = tci * (QC // P) + t4
                        psv = s1pool.tile([P, NH * HD], F32, name="ps_v", tag="s1")
                        for k in range(KO):
                            nc.tensor.matmul(
                                psv[:],
                                xts[tci][:, k, ts(t4, P)],
                                wv_sb[:, k, :],
                                start=(k == 0),
                                stop=(k == KO - 1),
                            )
                        nc.vector.tensor_add(
                            vex_sb[:, kb, :, :HD],
                            psv[:].rearrange("p (h d) -> p h d", h=NH),
                            bv_sb[:].rearrange("p (h d) -> p h d", h=NH),
                        )

                    return emit

                for m in range(2 * NH * HD // P):
                    groups.append(qk_group(m))
                for t4 in range(QC // P):
                    groups.append(v_group(t4))
                return groups

            def emit_pair(tci, pr, fillers=None):
                """Attention for head pair (2*pr, 2*pr+1) of q-chunk tci."""
                qt = qts[tci]
                nkb = (tci + 1) * (QC // P)
                fillers = list(fillers or [])
                held = fillers.pop() if len(fillers) > 1 else None
                fill_every = max(1, nkb // max(1, len(fillers))) if fillers else 0
                yac = yacpool.tile([P, 2, QC], F32, name="yac", tag="yac")
                pend = []  # deferred attV emits: (kb, q0, pt_tile)

                def attv(kb, q0, pt):
                    for pi in range(2):
                        nc.tensor.matmul(
                            yac[0 : HD + 1, pi, q0:QC],
                            vex_sb[:, kb, 2 * pr + pi, :],
                            pt[:, pi, q0:QC],
                            start=(kb == 0),
                            stop=(kb == nkb - 1),
                        )

                for kb in range(nkb):
                    d = kb - tci * (QC // P)
                    q0 = max(d, 0) * P
                    spair = scpool.tile([P, 2, QC], F32, name="spair", tag="sc")
                    for pi in range(2):
                        nc.tensor.matmul(
                            spair[:, pi, q0:QC],
                            kt_sb[ts(pi, HD), pr, ts(kb, P)],
                            qt[ts(pi, HD), pr, q0:QC],
                            start=True,
                            stop=True,
                            tile_position=(pi * HD, 0) if USE_TILEPOS else None,
                        )
                    pt = ptpool.tile([P, 2, QC], BF16, name="pt", tag="pt")
                    nc.scalar.activation(
                        pt[:, :, q0:QC],
                        spair[:, :, q0:QC],
                        mybir.ActivationFunctionType.Exp,
                        scale=1.0 / np.sqrt(HD),
                    )
                    if d >= 0:
                        for pi in range(2):
                            nc.vector.tensor_mul(
                                pt[:, pi, q0 : q0 + P],
                                pt[:, pi, q0 : q0 + P],
                                tri_sb[:],
                            )
                    pend.append((kb, q0, pt))
                    if len(pend) >= 2:
                        attv(*pend.pop(0))
                    if fillers and fill_every and kb % fill_every == fill_every - 1:
                        fillers.pop(0)()
                while pend:
                    attv(*pend.pop(0))
                for g in fillers:
                    g()

                # softmax normalization (baseline-proven pattern, per head):
                # reciprocal of the denominator row, PE ones-matmul broadcast
                # into a rotating s1 PSUM bank, evacuate, multiply.
                if held is not None:
                    held()
                rec = recpool.tile([1, 2, QC], F32, name="rec", tag="rec")
                nc.vector.reciprocal(rec[:], yac[HD : HD + 1, :, :])
                rec_r = recpool.tile([1, 2, QC], F32R, name="rec_r", tag="rec_r")
                nc.vector.tensor_copy(rec_r[:], rec[:])
                recb_ps = scpool.tile([P, 2, QC], F32, name="recb_ps", tag="sc")
                for pi in range(2):
                    nc.tensor.matmul(
                        recb_ps[0:HD, pi, :], ones_row[:], rec_r[:, pi, :],
                        start=True, stop=True,
                    )
                recb = rbpool.tile([HD, 2, QC], F32, name="recb", tag="recb")
                nc.vector.tensor_copy(recb[:], recb_ps[0:HD, :, :])
                for pi in range(2):
                    nc.vector.tensor_mul(
                        ytqs[tci][ts(pi, HD), pr, :],
                        yac[0:HD, pi, :],
                        recb[:, pi, :],
                    )

            ytqs = [None] * NQC

            def proj_groups(tci):
                groups = []

                def proj_m(m):
                    def emit():
                        pp = s1pool.tile([P, QC], F32, name="pp", tag="s1")
                        for kk in range(NH * HD // P):
                            nc.tensor.matmul(
                                pp[:],
                                wp_sb[:, kk, ts(m, P)],
                                ytqs[tci][:, kk, :],
                                start=(kk == 0),
                                stop=(kk == NH * HD // P - 1),
                            )
                        st = stpool.tile([P, QC], F32, name="st", tag="st")
                        nc.vector.tensor_copy(st[:], pp[:])
                        nc.sync.dma_start(yT_t[:, m, ts(tci, QC)], st[:])

                    return emit

                return [proj_m(m) for m in range(C // P)]

            # ---------------- schedule ----------------
            emit_xt_dma(0)
            for g in stage1_groups(0):
                g()
            for tci in range(NQC):
                ytqs[tci] = ytpool.tile(
                    [P, NH * HD // P, QC], BF16, name=f"ytq{tci}", tag="ytq"
                )
                fillers = []
                if tci + 1 < NQC:
                    emit_xt_dma(tci + 1)
                    fillers += stage1_groups(tci + 1)
                if tci > 0:
                    fillers += proj_groups(tci - 1)
                npr = NH // 2
                for pr in range(npr):
                    lo = (len(fillers) * pr) // npr
                    hi = (len(fillers) * (pr + 1)) // npr
                    emit_pair(tci, pr, fillers[lo:hi])
            for g in proj_groups(NQC - 1):
                g()

    return nc


def legalize_waits(nc):
    """This walrus build accepts at most 1 sync wait per instruction (0 for
    self-loading fp32/fp32r Matmult, whose LW slot takes none). Move excess
    waits onto preceding same-engine NoOps; engines execute in order so the
    guarantee is identical."""
    n = 0
    for blk in nc.m.functions[0].blocks:
        new = []
        for inst in blk.instructions:
            si = inst.sync_info
            waits = list(si.on_wait) if si is not None and si.on_wait else []
            lim = 0 if inst.opcode in ("Matmult", "Ldweights") else 1
            if len(waits) > lim:
                keep = waits[len(waits) - lim:] if lim else []
                for w in waits[: len(waits) - lim]:
                    n += 1
                    new.append(mybir.InstNoOp(
                        name=f"I-wfix{n}", engine=inst.engine, ins=[], outs=[],
                        sync_info=mybir.SyncInfo(on_wait=[w], on_update=[]),
                    ))
                inst.sync_info = mybir.SyncInfo(
                    on_wait=keep,
                    on_update=list(si.on_update) if si.on_update else [],
                )
            new.append(inst)
        blk.instructions = new
    return n


def _host_inputs(x, W_attn, b_attn, W_proj):
    """Build the 8 per-core input maps."""
    import ml_dtypes

    kl = np.arange(P)[:, None]
    ql = np.arange(P)[None, :]
    tri = (ql >= kl).astype(ml_dtypes.bfloat16)

    in_maps = []
    for core in range(8):
        b, g = core // 2, core % 2
        qs = slice(g * NH * HD, (g + 1) * NH * HD)
        ks = slice(C + g * NH * HD, C + (g + 1) * NH * HD)
        vs = slice(2 * C + g * NH * HD, 2 * C + (g + 1) * NH * HD)
        wqk = np.ascontiguousarray(
            np.concatenate([W_attn[:, qs], W_attn[:, ks]], axis=1)
        )
        bqk = (
            np.concatenate([b_attn[qs], b_attn[ks]])
            .reshape(2 * NH * HD // P, P)
            .T.copy()
        )
        in_maps.append(
            {
                "xT": np.ascontiguousarray(x[b].T),
                "Wqk": wqk,
                "Wv": np.ascontiguousarray(W_attn[:, vs]),
                "Wp": np.ascontiguousarray(
                    W_proj[g * NH * HD : (g + 1) * NH * HD]
                ).astype(ml_dtypes.bfloat16),
                "bqk": np.ascontiguousarray(bqk),
                "bv": np.ascontiguousarray(b_attn[vs]),
                "tri": tri,
                "one": np.ones([P], dtype=ml_dtypes.bfloat16),
            }
        )
    return in_maps


def run(x, W_attn, b_attn, W_proj, b_proj, trace=False):
    """Returns (y, BassKernelResults)."""
    x = np.asarray(x, dtype=np.float32)
    W_attn = np.asarray(W_attn, dtype=np.float32)
    b_attn = np.asarray(b_attn, dtype=np.float32)
    W_proj = np.asarray(W_proj, dtype=np.float32)
    b_proj = np.asarray(b_proj, dtype=np.float32)

    nc = build_nc()
    if os.environ.get("K2_NOLEGALIZE", "0") != "1":
        legalize_waits(nc)
    in_maps = _host_inputs(x, W_attn, b_attn, W_proj)
    res = run_bass_kernel_spmd(nc, in_maps, list(range(8)), trace=trace)

    y = np.empty((B, T, C), dtype=np.float32)
    for b in range(B):
        acc = res.results[2 * b]["yT"] + res.results[2 * b + 1]["yT"]
        y[b] = acc.T + b_proj
    return y, res


def kernel(x, W_attn, b_attn, W_proj, b_proj):
    y, _ = run(x, W_attn, b_attn, W_proj, b_proj)
    return y
